# revision 1
# baseline (speedup 1.0000x reference)
"""Cross-attention kernel for 8 trn2 NeuronCores.

Reference computation (per batch b of 16):
  q = Wq @ x, k = Wk @ y, v = Wv @ y          (1x1 convs as channel matmuls)
  q,k l2-normalized over the SPATIAL axis (per (h,d) row)
  sim = 10 * q^T k per head; attn = softmax_j(sim); o = attn @ v^T
  out = Wo @ o + b

Sharding: data-parallel over batch, 2 batches per core, weights replicated.

Per-core kernel layout choices:
  - q,k kept in [head*d, n] layout (partition = channel rows) so the l2 norm
    is a free-axis reduction; the combined scale 1/(||q_d||*||k_d||) is folded
    into k only, and the *10 into the exp()'s free affine.
  - sim computed TRANSPOSED: S_T[j, i] = sum_d k[d,j] q[d,i], so the softmax
    denominator (sum over j) is a partition-axis sum, done on the PE by
    appending an all-ones column to v^T: row 64 of the PV accumulation is
    sum_j exp(S_T[j,i]).  exp needs no max-subtraction (|sim|<=0.6 for these
    inputs, verified).
  - v produced directly in transposed layout [j, (head,d)+ones] from the PE
    (y tile used as lhsT), interleaved as [v_h | 1] blocks of 65 columns so
    each head's PV lhsT is one contiguous [128, 65] slice.
  - normalization: denominator row reshaped [1,1024]->[128,8] via SBUF DMA,
    reciprocal on DVE, reshaped back, partition_broadcast on GPSIMD, then one
    tensor_tensor multiply PSUM->SBUF.
  - all matmuls in float32r (full PE rate at N>=256), fp32 storage/accum.
"""

import sys

import numpy as np

if "/opt/trn_rl_repo" not in sys.path:
    sys.path.insert(0, "/opt/trn_rl_repo")

NB = 2        # batches per core
C = 256       # channels
N = 1024      # spatial (32*32)
HEADS = 4
DH = 64
HID = 256
NCORES = 8
MAGIC = 0x5F3759DF  # Quake fast inverse-sqrt seed

_CACHE = {}


def _quake_rsqrt(nc, pool, p_ap, out_ap, final_scale):
    """out = rsqrt(p) * final_scale for [128,1] fp32 APs, DVE-only.

    Quake seed + 2 Newton iterations (rel err ~1e-7), no ACT table needed.
    """
    from concourse import mybir

    i32 = mybir.dt.int32
    alu = mybir.AluOpType
    t = pool.tile([128, 1], mybir.dt.float32, tag="qk_rs_t", bufs=4)
    r = pool.tile([128, 1], mybir.dt.float32, tag="qk_rs_r", bufs=4)
    a = pool.tile([128, 1], mybir.dt.float32, tag="qk_rs_a", bufs=4)
    # seed: r0 = bitcast(MAGIC - (bitcast_i32(p) >> 1))
    nc.vector.tensor_scalar(t.bitcast(i32), p_ap.bitcast(i32), 1, None,
                            alu.logical_shift_right)
    nc.vector.tensor_scalar(r.bitcast(i32), t.bitcast(i32), -1, MAGIC,
                            alu.mult, alu.add)
    # Newton 1: r = r * (1.5 - 0.5 * p * r^2)
    nc.vector.scalar_tensor_tensor(a[:], r[:], r[:, 0:1], p_ap,
                                   alu.mult, alu.mult)
    nc.vector.tensor_scalar(a[:], a[:], -0.5, 1.5, alu.mult, alu.add)
    nc.vector.tensor_scalar(t[:], a[:], r[:, 0:1], None, alu.mult)
    # Newton 2 (fold final_scale into the last multiply)
    nc.vector.scalar_tensor_tensor(a[:], t[:], t[:, 0:1], p_ap,
                                   alu.mult, alu.mult)
    nc.vector.tensor_scalar(a[:], a[:], -0.5, 1.5, alu.mult, alu.add)
    nc.vector.tensor_scalar(out_ap, a[:], t[:, 0:1], final_scale,
                            alu.mult, alu.mult)


def _build_nc():
    from contextlib import ExitStack

    import concourse.tile as tile
    from concourse import bacc, mybir

    f32 = mybir.dt.float32
    f16 = mybir.dt.float16
    alu = mybir.AluOpType
    EXP = mybir.ActivationFunctionType.Exp

    nc = bacc.Bacc("TRN2", target_bir_lowering=False)

    xin = nc.dram_tensor("x", [NB, C, N], f16, kind="ExternalInput")
    yin = nc.dram_tensor("y", [NB, C, N], f16, kind="ExternalInput")
    wq = nc.dram_tensor("wq_t", [C, HID], f16, kind="ExternalInput")
    wk = nc.dram_tensor("wk_t", [C, HID], f16, kind="ExternalInput")
    wv = nc.dram_tensor("wv_t", [C, HID], f16, kind="ExternalInput")
    wo = nc.dram_tensor("wo_t", [HID, C], f16, kind="ExternalInput")
    bo = nc.dram_tensor("b_out", [2, 128, 1], f32, kind="ExternalInput")
    out = nc.dram_tensor("out", [NB, C, N], f32, kind="ExternalOutput")

    with tile.TileContext(nc) as tc, ExitStack() as ctx:
        consts = ctx.enter_context(tc.tile_pool(name="consts", bufs=1))
        big = ctx.enter_context(tc.tile_pool(name="big", bufs=2))
        sm = ctx.enter_context(tc.tile_pool(name="sm", bufs=4))
        ps = ctx.enter_context(tc.tile_pool(name="ps", bufs=2, space="PSUM"))

        # ---- input + weight loads (batch-0 x/y first: critical path) --
        wq_sb = consts.tile([128, 2, HID], f16, tag="wq")
        wk_sb = consts.tile([128, 2, HID], f16, tag="wk")
        wv_sb = consts.tile([128, 2, HID], f16, tag="wv")
        wo_sb = consts.tile([128, 2, C], f16, tag="wo")
        b_sb = consts.tile([128, 2, 1], f32, tag="bo")
        # warm the ACT exp table while input DMAs are in flight
        warm = sm.tile([128, 1], f32, tag="warm", bufs=1)
        nc.vector.memset(warm[:], 0.0)
        nc.scalar.activation(out=warm[:], in_=warm[:], func=EXP, scale=1.0)
        xts, yts = [], []
        for nb in range(NB):
            xt = big.tile([128, 2, N], f16, tag="xt", bufs=2)
            yt = big.tile([128, 2, N], f16, tag="yt", bufs=2)
            xts.append(xt)
            yts.append(yt)
        nc.sync.dma_start(out=yts[0][:], in_=yin[0].rearrange("(kc p) n -> p kc n", p=128))
        nc.sync.dma_start(out=xts[0][:], in_=xin[0].rearrange("(kc p) n -> p kc n", p=128))
        nc.sync.dma_start(out=wk_sb[:], in_=wk.rearrange("(kc p) n -> p kc n", p=128))
        nc.sync.dma_start(out=wq_sb[:], in_=wq.rearrange("(kc p) n -> p kc n", p=128))
        nc.sync.dma_start(out=wv_sb[:], in_=wv.rearrange("(kc p) n -> p kc n", p=128))
        nc.sync.dma_start(out=wo_sb[:], in_=wo.rearrange("(kc p) n -> p kc n", p=128))
        nc.sync.dma_start(out=b_sb[:], in_=bo.rearrange("kc p n -> p kc n"))
        nc.sync.dma_start(out=xts[1][:], in_=xin[1].rearrange("(kc p) n -> p kc n", p=128))
        nc.sync.dma_start(out=yts[1][:], in_=yin[1].rearrange("(kc p) n -> p kc n", p=128))

        # ---- per-batch stages, emitted interleaved so the Tile pool
        # FIFOs (psum tag ps_big / SBUF tags) line up with the desired
        # cross-batch overlap: batch-1 projections fill PE gaps while the
        # ACT engine streams batch-0 exps.
        def proj_qk(nb, qn, kn, kraw, mc):
            kp = ps.tile([128, N], f32, tag="ps_acc", bufs=2)
            qp = ps.tile([128, N], f32, tag="ps_acc", bufs=2)
            for ih in range(2):
                for kc in range(2):
                    nc.tensor.matmul(
                        kp[:, ih * 512:(ih + 1) * 512],
                        wk_sb[:, kc, mc * 128:(mc + 1) * 128],
                        yts[nb][:, kc, ih * 512:(ih + 1) * 512],
                        start=(kc == 0), stop=(kc == 1))
            for ih in range(2):
                for kc in range(2):
                    nc.tensor.matmul(
                        qp[:, ih * 512:(ih + 1) * 512],
                        wq_sb[:, kc, mc * 128:(mc + 1) * 128],
                        xts[nb][:, kc, ih * 512:(ih + 1) * 512],
                        start=(kc == 0), stop=(kc == 1))
            # k first: kn gates the first S_T matmul (LDW side)
            nc.vector.tensor_copy(kraw[:, mc, :], kp[:])
            nc.vector.tensor_copy(qn[:, mc, :], qp[:])
            stq = sm.tile([128, 2, 6], f32, tag="stq", bufs=4)
            stk = sm.tile([128, 2, 6], f32, tag="stk", bufs=4)
            mvq = sm.tile([128, 2], f32, tag="mvq", bufs=4)
            mvk = sm.tile([128, 2], f32, tag="mvk", bufs=4)
            for sub in range(2):
                nc.vector.bn_stats(out=stk[:, sub, :], in_=kraw[:, mc, sub * 512:(sub + 1) * 512])
                nc.vector.bn_stats(out=stq[:, sub, :], in_=qn[:, mc, sub * 512:(sub + 1) * 512])
            nc.vector.bn_aggr(out=mvk[:], in_=stk[:])
            nc.vector.bn_aggr(out=mvq[:], in_=stq[:])
            uq = sm.tile([128, 1], f32, tag="uq", bufs=4)
            uk = sm.tile([128, 1], f32, tag="uk", bufs=4)
            pqk = sm.tile([128, 1], f32, tag="pqk", bufs=4)
            nc.vector.scalar_tensor_tensor(uq[:], mvq[:, 0:1], mvq[:, 0:1],
                                           mvq[:, 1:2], alu.mult, alu.add)
            nc.vector.scalar_tensor_tensor(uk[:], mvk[:, 0:1], mvk[:, 0:1],
                                           mvk[:, 1:2], alu.mult, alu.add)
            nc.vector.tensor_mul(pqk[:], uq[:], uk[:])
            sck = sm.tile([128, 1], f32, tag="sck", bufs=4)
            _quake_rsqrt(nc, sm, pqk[:], sck[:], 1.0 / float(N))
            nc.vector.tensor_scalar(kn[:, mc, :], kraw[:, mc, :], sck[:, 0:1], None, alu.mult)

        def proj_v(nb, jcs, vts_nb):
            for jc in jcs:
                vp = ps.tile([128, N], f32, tag="ps_acc", bufs=2)
                for kc in range(2):
                    nc.tensor.matmul(
                        vp[:, 0:HID],
                        yts[nb][:, kc, jc * 128:(jc + 1) * 128],
                        wv_sb[:, kc, :],
                        start=(kc == 0), stop=(kc == 1))
                vt = big.tile([128, 4, 65], f16, tag="vt", bufs=16)
                nc.vector.tensor_copy(vt[:, :, 0:64],
                                      vp[:, 0:HID].rearrange("p (h d) -> p h d", h=4))
                nc.gpsimd.memset(vt[:, :, 64:65], 1.0)
                vts_nb.append(vt)

        def attn_head(nb, h, qn, kn, vts_nb, o_sb):
            hp, hr = h // 2, 64 * (h % 2)
            op = ps.tile([128, N], f32, tag="ps_acc", bufs=2)
            for jc in range(8):
                st = ps.tile([128, N], f32, tag="ps_big", bufs=2)
                for ih in range(2):
                    nc.tensor.matmul(
                        st[:, ih * 512:(ih + 1) * 512],
                        kn[hr:hr + 64, hp, jc * 128:(jc + 1) * 128],
                        qn[hr:hr + 64, hp, ih * 512:(ih + 1) * 512],
                        start=True, stop=True)
                et = big.tile([128, N], f16, tag="et", bufs=8)
                nc.scalar.activation(out=et[:], in_=st[:], func=EXP, scale=10.0)
                vt = vts_nb[jc]
                for ih in range(2):
                    nc.tensor.matmul(
                        op[0:65, ih * 512:(ih + 1) * 512],
                        vt[:, h, :],
                        et[:, ih * 512:(ih + 1) * 512],
                        start=(jc == 0), stop=(jc == 7))
            # normalization: row 64 of op is the softmax denominator
            srow = sm.tile([1, N], f32, tag="srow", bufs=4)
            s2 = sm.tile([128, 8], f32, tag="s2", bufs=4)
            r2 = sm.tile([128, 8], f32, tag="r2", bufs=4)
            rrow = sm.tile([1, N], f32, tag="rrow", bufs=4)
            rb = big.tile([64, N], f32, tag="rb", bufs=4)
            nc.vector.tensor_copy(srow[:], op[64:65, :])
            nc.sync.dma_start(out=s2[:], in_=srow[:])
            nc.vector.reciprocal(r2[:], s2[:])
            nc.sync.dma_start(out=rrow[:], in_=r2[:])
            nc.gpsimd.partition_broadcast(rb[:], rrow[:])
            nc.vector.tensor_mul(o_sb[hr:hr + 64, hp, :], op[0:64, :], rb[:])

        def zproj(nb, o_sb):
            for mc in range(2):
                zp = ps.tile([128, N], f32, tag="ps_acc", bufs=2)
                for ih in range(2):
                    for kc in range(2):
                        nc.tensor.matmul(
                            zp[:, ih * 512:(ih + 1) * 512],
                            wo_sb[:, kc, mc * 128:(mc + 1) * 128],
                            o_sb[:, kc, ih * 512:(ih + 1) * 512],
                            start=(kc == 0), stop=(kc == 1))
                zs = big.tile([128, N], f32, tag="zs", bufs=4)
                nc.vector.tensor_scalar(zs[:], zp[:], b_sb[:, mc, 0:1], None, alu.add)
                nc.sync.dma_start(out=out[nb, mc * 128:(mc + 1) * 128, :], in_=zs[:])

        def alloc_qk():
            qn = big.tile([128, 2, N], f16, tag="qn", bufs=2)
            kn = big.tile([128, 2, N], f16, tag="kn", bufs=2)
            kraw = big.tile([128, 2, N], f16, tag="kraw", bufs=2)
            return qn, kn, kraw

        def alloc_o():
            o_sb = big.tile([128, 2, N], f16, tag="osb", bufs=2)
            return o_sb

        # batch 0: only the mc0 q/k chunk gates head 0; everything else
        # rides the ps_acc FIFO off the ACT-critical path
        qn0, kn0, kraw0 = alloc_qk()
        vts0 = []
        proj_qk(0, qn0, kn0, kraw0, 0)
        proj_v(0, range(8), vts0)
        o0 = alloc_o()
        qn1, kn1, kraw1 = alloc_qk()
        vts1 = []
        attn_head(0, 0, qn0, kn0, vts0, o0)
        proj_qk(0, qn0, kn0, kraw0, 1)
        attn_head(0, 1, qn0, kn0, vts0, o0)
        proj_qk(1, qn1, kn1, kraw1, 0)
        attn_head(0, 2, qn0, kn0, vts0, o0)
        proj_qk(1, qn1, kn1, kraw1, 1)
        attn_head(0, 3, qn0, kn0, vts0, o0)
        proj_v(1, range(0, 4), vts1)
        o1 = alloc_o()
        proj_v(1, range(4, 8), vts1)
        attn_head(1, 0, qn1, kn1, vts1, o1)
        zproj(0, o0)
        attn_head(1, 1, qn1, kn1, vts1, o1)
        attn_head(1, 2, qn1, kn1, vts1, o1)
        attn_head(1, 3, qn1, kn1, vts1, o1)
        zproj(1, o1)

    nc.finalize()
    return nc


def _get_nc():
    if "nc" not in _CACHE:
        _CACHE["nc"] = _build_nc()
    return _CACHE["nc"]


def kernel(x, y, w_qkv, w_out, b_out):
    from concourse.bass_utils import run_bass_kernel_spmd

    nc = _get_nc()

    x = np.asarray(x, dtype=np.float32).reshape(16, C, N).astype(np.float16)
    y = np.asarray(y, dtype=np.float32).reshape(16, C, N).astype(np.float16)
    w_qkv = np.asarray(w_qkv, dtype=np.float32)
    wq_t = np.ascontiguousarray(w_qkv[0:HID].T).astype(np.float16)
    wk_t = np.ascontiguousarray(w_qkv[HID:2 * HID].T).astype(np.float16)
    wv_t = np.ascontiguousarray(w_qkv[2 * HID:3 * HID].T).astype(np.float16)
    wo_t = np.ascontiguousarray(np.asarray(w_out, dtype=np.float32).T).astype(np.float16)
    bo = np.ascontiguousarray(
        np.asarray(b_out, dtype=np.float32).reshape(2, 128, 1))

    in_maps = []
    for c in range(NCORES):
        in_maps.append({
            "x": np.ascontiguousarray(x[c * NB:(c + 1) * NB]),
            "y": np.ascontiguousarray(y[c * NB:(c + 1) * NB]),
            "wq_t": wq_t, "wk_t": wk_t, "wv_t": wv_t, "wo_t": wo_t,
            "b_out": bo,
        })

    res = run_bass_kernel_spmd(nc, in_maps, list(range(NCORES)))
    full = np.concatenate([res.results[i]["out"] for i in range(NCORES)], axis=0)
    return full.reshape(16, C, 32, 32)



# revision 5
# speedup vs baseline: 1.1559x; 1.1559x over previous
"""Cross-attention kernel for 8 trn2 NeuronCores.

Reference computation (per batch b of 16):
  q = Wq @ x, k = Wk @ y, v = Wv @ y          (1x1 convs as channel matmuls)
  q,k l2-normalized over the SPATIAL axis (per (h,d) row)
  sim = 10 * q^T k per head; attn = softmax_j(sim); o = attn @ v^T
  out = Wo @ o + b

Sharding: data-parallel over batch, 2 batches per core, weights replicated.

Design (v2 — ACT-saturating schedule):
  - The softmax exp stream on the Activation engine is the hard floor
    (64 x [128,1024] exps ~= 66us); everything else hides under it.
  - S_T[j,i] = sum_d k8[d,j] q8[d,i] computed in fp8e4 DoubleRow
    (contraction 2x32 over partition pairs, 0.5 cyc/row): q8 carries the
    combined norm scale CQ/(||q||*||k||), k8 = k/4 raw; exp scale 10*4/CQ.
    q8/k8 are DMA-regrouped to [32, head, half, N] layout for the
    DoubleRow [K,2,M]/[K,2,N] access patterns.
  - Emission is software-pipelined: S_T(u+2) is emitted after PV(u) so the
    in-order PE queue never blocks the next exp behind a PV waiting on the
    current exp (the v1 kernel serialized here).
  - PV stays fp16 (vt = [j, (h d)+ones] from the PE, et = exp in fp16):
    fp8 PV fails the 2e-2 gate (~5% error).
  - softmax denominator: ones-column in vt gives row 64 of the PV psum;
    reciprocal directly on the [1,512] halves, gpsimd partition_broadcast,
    one tensor_mul per half. z-proj + bias split by i-halves at the end to
    pipeline the epilogue.
"""

import sys

import numpy as np

if "/opt/trn_rl_repo" not in sys.path:
    sys.path.insert(0, "/opt/trn_rl_repo")

NB = 2        # batches per core
C = 256       # channels
N = 1024      # spatial (32*32)
HEADS = 4
DH = 64
HID = 256
NCORES = 8
MAGIC = 0x5F3759DF  # Quake fast inverse-sqrt seed
CQ = 32768.0        # combined-norm scale folded into q8 (k8 = k/4)
EXP_SCALE = 10.0 * 4.0 / CQ

_CACHE = {}


def _quake_rsqrt(nc, pool, p_ap, out_ap, final_scale):
    """out = rsqrt(p) * final_scale for [128,1] fp32 APs, DVE-only."""
    from concourse import mybir

    i32 = mybir.dt.int32
    alu = mybir.AluOpType
    t = pool.tile([128, 1], mybir.dt.float32, tag="qk_rs_t", bufs=4)
    r = pool.tile([128, 1], mybir.dt.float32, tag="qk_rs_r", bufs=4)
    a = pool.tile([128, 1], mybir.dt.float32, tag="qk_rs_a", bufs=4)
    nc.vector.tensor_scalar(t.bitcast(i32), p_ap.bitcast(i32), 1, None,
                            alu.logical_shift_right)
    nc.vector.tensor_scalar(r.bitcast(i32), t.bitcast(i32), -1, MAGIC,
                            alu.mult, alu.add)
    nc.vector.scalar_tensor_tensor(a[:], r[:], r[:, 0:1], p_ap,
                                   alu.mult, alu.mult)
    nc.vector.tensor_scalar(a[:], a[:], -0.5, 1.5, alu.mult, alu.add)
    nc.vector.tensor_scalar(t[:], a[:], r[:, 0:1], None, alu.mult)
    nc.vector.scalar_tensor_tensor(a[:], t[:], t[:, 0:1], p_ap,
                                   alu.mult, alu.mult)
    nc.vector.tensor_scalar(a[:], a[:], -0.5, 1.5, alu.mult, alu.add)
    nc.vector.tensor_scalar(out_ap, a[:], t[:, 0:1], final_scale,
                            alu.mult, alu.mult)


def _build_nc():
    from contextlib import ExitStack

    import concourse.tile as tile
    from concourse import bacc, mybir

    f32 = mybir.dt.float32
    f16 = mybir.dt.float16
    f8 = mybir.dt.float8e4
    alu = mybir.AluOpType
    EXP = mybir.ActivationFunctionType.Exp
    DR = mybir.MatmulPerfMode.DoubleRow

    nc = bacc.Bacc("TRN2", target_bir_lowering=False)

    xin = nc.dram_tensor("x", [NB, C, N], f16, kind="ExternalInput")
    yin = nc.dram_tensor("y", [NB, C, N], f16, kind="ExternalInput")
    wq = nc.dram_tensor("wq_t", [C, HID], f16, kind="ExternalInput")
    wk = nc.dram_tensor("wk_t", [C, HID], f16, kind="ExternalInput")
    wv = nc.dram_tensor("wv_t", [C, HID], f16, kind="ExternalInput")
    wo = nc.dram_tensor("wo_t", [HID, C], f16, kind="ExternalInput")
    bo = nc.dram_tensor("b_out", [2, 128, 1], f32, kind="ExternalInput")
    out = nc.dram_tensor("out", [NB, C, N], f16, kind="ExternalOutput")

    with tile.TileContext(nc) as tc, ExitStack() as ctx:
        consts = ctx.enter_context(tc.tile_pool(name="consts", bufs=1))
        big = ctx.enter_context(tc.tile_pool(name="big", bufs=2))
        sm = ctx.enter_context(tc.tile_pool(name="sm", bufs=4))
        ps = ctx.enter_context(tc.tile_pool(name="ps", bufs=2, space="PSUM"))

        wq_sb = consts.tile([128, 2, HID], f16, tag="wq")
        wk_sb = consts.tile([128, 2, HID], f16, tag="wk")
        wv_sb = consts.tile([128, 2, HID], f16, tag="wv")
        wo_sb = consts.tile([128, 2, C], f16, tag="wo")
        b_sb = consts.tile([128, 2, 1], f32, tag="bo")
        # warm the ACT exp table while input DMAs are in flight
        warm = sm.tile([128, 1], f32, tag="warm", bufs=1)
        nc.vector.memset(warm[:], 0.0)
        nc.scalar.activation(out=warm[:], in_=warm[:], func=EXP, scale=1.0)

        xts, yts = [], []
        for nb in range(NB):
            xts.append(big.tile([128, 2, N], f16, tag="xt", bufs=2, name=f"xt{nb}"))
            yts.append(big.tile([128, 2, N], f16, tag="yt", bufs=2, name=f"yt{nb}"))
        nc.sync.dma_start(out=wk_sb[:], in_=wk.rearrange("(kc p) n -> p kc n", p=128))
        nc.sync.dma_start(out=wq_sb[:], in_=wq.rearrange("(kc p) n -> p kc n", p=128))
        nc.sync.dma_start(out=yts[0][:], in_=yin[0].rearrange("(kc p) n -> p kc n", p=128))
        nc.sync.dma_start(out=xts[0][:], in_=xin[0].rearrange("(kc p) n -> p kc n", p=128))
        nc.sync.dma_start(out=wv_sb[:], in_=wv.rearrange("(kc p) n -> p kc n", p=128))
        nc.sync.dma_start(out=wo_sb[:], in_=wo.rearrange("(kc p) n -> p kc n", p=128))
        nc.sync.dma_start(out=b_sb[:], in_=bo.rearrange("kc p n -> p kc n"))
        nc.sync.dma_start(out=yts[1][:], in_=yin[1].rearrange("(kc p) n -> p kc n", p=128))
        nc.sync.dma_start(out=xts[1][:], in_=xin[1].rearrange("(kc p) n -> p kc n", p=128))

        # ---------------- state ----------------
        units = [(b, h, jc) for b in range(NB) for h in range(4) for jc in range(8)]
        st_tiles = {}
        et_tiles = {}
        op_tiles = {}
        kq_psum = {}     # (b, mc, 'k'|'q') -> psum tile
        nrm = {}         # (b, mc, 'k'|'q') -> E[x^2] [128,1]
        sq_scale = {}    # (b, mc) -> combined scale [128,1]
        kn8t = {}        # (b, mc, 'k'|'q') -> [128, N] fp8 flat
        qs8 = {}         # (b, mc) -> [32, 2, 2, N] fp8 regrouped (q)
        ks8 = {}
        vts = {b: {} for b in range(NB)}
        o_tiles = {}

        def emit_kq_mm(b, mc, t):
            """projection matmuls for tensor t ('k' from y / 'q' from x)."""
            w_sb = wk_sb if t == "k" else wq_sb
            src = yts[b] if t == "k" else xts[b]
            p = ps.tile([128, N], f32, tag="ps_acc", bufs=2, name=f"kqp_{b}{mc}{t}")
            for ih in range(2):
                for kc in range(2):
                    nc.tensor.matmul(
                        p[:, ih * 512:(ih + 1) * 512],
                        w_sb[:, kc, mc * 128:(mc + 1) * 128],
                        src[:, kc, ih * 512:(ih + 1) * 512],
                        start=(kc == 0), stop=(kc == 1))
            kq_psum[(b, mc, t)] = p

        def emit_bn(b, mc, t):
            p = kq_psum[(b, mc, t)]
            st = sm.tile([128, 2, 6], f32, tag="bnst", bufs=8)
            mv = sm.tile([128, 2], f32, tag="bnmv", bufs=8)
            u = sm.tile([128, 1], f32, tag="bnu", bufs=8)
            for sub in range(2):
                nc.vector.bn_stats(out=st[:, sub, :], in_=p[:, sub * 512:(sub + 1) * 512])
            nc.vector.bn_aggr(out=mv[:], in_=st[:])
            nc.vector.scalar_tensor_tensor(u[:], mv[:, 0:1], mv[:, 0:1],
                                           mv[:, 1:2], alu.mult, alu.add)
            nrm[(b, mc, t)] = u

        def emit_quake(b, mc):
            pq = sm.tile([128, 1], f32, tag="pqk", bufs=4)
            sc = sm.tile([128, 1], f32, tag="sck", bufs=4)
            nc.vector.tensor_mul(pq[:], nrm[(b, mc, "k")][:], nrm[(b, mc, "q")][:])
            # s = CQ / (||q|| * ||k||) = (CQ/N) * rsqrt(Eq*Ek)
            _quake_rsqrt(nc, sm, pq[:], sc[:], CQ / N)
            sq_scale[(b, mc)] = sc

        def emit_conv_raw(b, mc):
            p = kq_psum.pop((b, mc, "k"))
            k8 = big.tile([128, N], f8, tag="kn8", bufs=2)
            nc.vector.tensor_scalar(k8[:], p[:], 0.25, None, alu.mult)
            kn8t[(b, mc, "k")] = k8

        def emit_conv_fold(b, mc):
            p = kq_psum.pop((b, mc, "q"))
            q8 = big.tile([128, N], f8, tag="qn8", bufs=2)
            nc.vector.tensor_scalar(q8[:], p[:], sq_scale[(b, mc)][:, 0:1],
                                    None, alu.mult)
            kn8t[(b, mc, "q")] = q8

        def emit_regroup(b, mc, t):
            src = kn8t.pop((b, mc, t))
            dst = big.tile([32, 2, 2, N], f8, tag="qs8" if t == "q" else "ks8",
                           bufs=4)
            for h2 in range(2):
                for m in range(2):
                    p0 = h2 * 64 + m * 32
                    nc.sync.dma_start(out=dst[:, h2, m, :], in_=src[p0:p0 + 32, :])
            (qs8 if t == "q" else ks8)[(b, mc)] = dst

        def emit_vp(b, jc):
            vp = ps.tile([128, N], f32, tag="ps_acc", bufs=2)
            for kc in range(2):
                nc.tensor.matmul(
                    vp[:, 0:HID],
                    yts[b][:, kc, jc * 128:(jc + 1) * 128],
                    wv_sb[:, kc, :],
                    start=(kc == 0), stop=(kc == 1))
            vt = big.tile([128, 4, 65], f16, tag="vt", bufs=16)
            nc.vector.tensor_copy(vt[:, :, 0:64],
                                  vp[:, 0:HID].rearrange("p (h d) -> p h d", h=4))
            nc.gpsimd.memset(vt[:, :, 64:65], 1.0)
            vts[b][jc] = vt

        def emit_st(u):
            b, h, jc = units[u]
            hp, hr = h // 2, h % 2
            st = ps.tile([128, N], f32, tag="ps_big", bufs=2)
            kt = ks8[(b, hp)]
            qt = qs8[(b, hp)]
            for ic in range(4):
                nc.tensor.matmul(
                    st[:, ic * 256:(ic + 1) * 256],
                    kt[:, hr, :, jc * 128:(jc + 1) * 128],
                    qt[:, hr, :, ic * 256:(ic + 1) * 256],
                    start=True, stop=True, perf_mode=DR)
            st_tiles[u] = st

        def emit_exp(u):
            et = big.tile([128, N], f16, tag="et", bufs=4)
            nc.scalar.activation(out=et[:], in_=st_tiles.pop(u)[:], func=EXP,
                                 scale=EXP_SCALE)
            et_tiles[u] = et

        def emit_tail(b, h):
            hp, hr = h // 2, 64 * (h % 2)
            op = op_tiles.pop((b, h))
            rr = sm.tile([1, N], f32, tag="rr", bufs=2)
            rbt = big.tile([64, N], f32, tag="rb", bufs=2)
            for ih in range(2):
                sl = slice(ih * 512, (ih + 1) * 512)
                nc.vector.reciprocal(rr[0:1, sl], op[64:65, sl])
            for ih in range(2):
                sl = slice(ih * 512, (ih + 1) * 512)
                nc.gpsimd.partition_broadcast(rbt[:, sl], rr[0:1, sl])
            for ih in range(2):
                sl = slice(ih * 512, (ih + 1) * 512)
                nc.vector.tensor_mul(o_tiles[b][hr:hr + 64, hp, sl],
                                     op[0:64, sl], rbt[:, sl])

        def emit_pv(u):
            b, h, jc = units[u]
            if jc == 0:
                op_tiles[(b, h)] = ps.tile([128, N], f32, tag="ps_acc", bufs=2, name=f"op_{b}{h}")
            op = op_tiles[(b, h)]
            et = et_tiles.pop(u)
            vt = vts[b][jc]
            for ih in range(2):
                nc.tensor.matmul(
                    op[0:65, ih * 512:(ih + 1) * 512],
                    vt[:, h, :],
                    et[:, ih * 512:(ih + 1) * 512],
                    start=(jc == 0), stop=(jc == 7))
            if jc == 7:
                emit_tail(b, h)

        def emit_z(b, mc):
            zp = ps.tile([128, N], f32, tag="ps_acc", bufs=2, name=f"zp_{b}{mc}")
            for ih in range(2):
                for kc in range(2):
                    nc.tensor.matmul(
                        zp[:, ih * 512:(ih + 1) * 512],
                        wo_sb[:, kc, mc * 128:(mc + 1) * 128],
                        o_tiles[b][:, kc, ih * 512:(ih + 1) * 512],
                        start=(kc == 0), stop=(kc == 1))
            zs = big.tile([128, N], f16, tag="zs", bufs=4)
            nc.vector.tensor_scalar(zs[:], zp[:], b_sb[:, mc, 0:1], None, alu.add)
            nc.sync.dma_start(out=out[b, mc * 128:(mc + 1) * 128, :], in_=zs[:])

        def emit_z_final(b):
            """ih-split z for the last batch: pipeline PE/DVE/Pool/DMA."""
            zps = [ps.tile([128, N], f32, tag="ps_acc", bufs=2, name=f"zpf{mc}") for mc in range(2)]
            zss = [big.tile([128, N], f16, tag="zs", bufs=4, name=f"zsf{mc}") for mc in range(2)]
            for ih in range(2):
                sl = slice(ih * 512, (ih + 1) * 512)
                for mc in range(2):
                    for kc in range(2):
                        nc.tensor.matmul(
                            zps[mc][:, sl],
                            wo_sb[:, kc, mc * 128:(mc + 1) * 128],
                            o_tiles[b][:, kc, sl],
                            start=(kc == 0), stop=(kc == 1))
                nc.vector.tensor_scalar(zss[0][:, sl], zps[0][:, sl],
                                        b_sb[:, 0, 0:1], None, alu.add)
                nc.vector.tensor_scalar(zss[1][:, sl], zps[1][:, sl],
                                        b_sb[:, 1, 0:1], None, alu.add)
                for mc in range(2):
                    nc.sync.dma_start(out=out[b, mc * 128:(mc + 1) * 128, sl],
                                      in_=zss[mc][:, sl])

        # ---------------- prologue (batch 0) ----------------
        o_tiles[0] = big.tile([128, 2, N], f16, tag="osb", bufs=2, name="osb0")
        emit_kq_mm(0, 0, "k")
        emit_kq_mm(0, 0, "q")
        emit_bn(0, 0, "k")
        emit_conv_raw(0, 0)
        emit_regroup(0, 0, "k")
        emit_bn(0, 0, "q")
        emit_quake(0, 0)
        emit_conv_fold(0, 0)
        emit_regroup(0, 0, "q")
        for jc in range(8):
            emit_vp(0, jc)
        emit_kq_mm(0, 1, "k")
        emit_kq_mm(0, 1, "q")
        emit_bn(0, 1, "k")
        emit_conv_raw(0, 1)
        emit_regroup(0, 1, "k")
        emit_bn(0, 1, "q")
        emit_quake(0, 1)
        emit_conv_fold(0, 1)
        emit_regroup(0, 1, "q")

        # ---------------- pipelined attention units ----------------
        def ins_o1():
            o_tiles[1] = big.tile([128, 2, N], f16, tag="osb", bufs=2, name="osb1")

        insertions = {
            8: [lambda: emit_kq_mm(1, 0, "k")],
            9: [lambda: emit_bn(1, 0, "k"), lambda: emit_conv_raw(1, 0)],
            10: [lambda: emit_kq_mm(1, 0, "q")],
            11: [lambda: emit_bn(1, 0, "q"), lambda: emit_quake(1, 0)],
            12: [lambda: emit_conv_fold(1, 0)],
            13: [lambda: emit_regroup(1, 0, "k")],
            14: [lambda: emit_regroup(1, 0, "q")],
            17: [lambda: emit_kq_mm(1, 1, "k")],
            18: [lambda: emit_bn(1, 1, "k"), lambda: emit_conv_raw(1, 1)],
            19: [lambda: emit_kq_mm(1, 1, "q")],
            20: [lambda: emit_bn(1, 1, "q"), lambda: emit_quake(1, 1)],
            21: [lambda: emit_conv_fold(1, 1)],
            22: [lambda: emit_regroup(1, 1, "k")],
            23: [lambda: emit_regroup(1, 1, "q"), ins_o1],
            24: [lambda: emit_vp(1, 0)],
            25: [lambda: emit_vp(1, 1)],
            26: [lambda: emit_vp(1, 2)],
            27: [lambda: emit_vp(1, 3)],
            28: [lambda: emit_vp(1, 4)],
            29: [lambda: emit_vp(1, 5)],
            30: [lambda: emit_vp(1, 6)],
            31: [lambda: emit_vp(1, 7)],
            32: [lambda: emit_z(0, 0)],
            34: [lambda: emit_z(0, 1)],
        }

        emit_st(0)
        emit_st(1)
        for u in range(64):
            emit_exp(u)
            emit_pv(u)
            for f in insertions.get(u, []):
                f()
            if u + 2 < 64:
                emit_st(u + 2)
        emit_z_final(1)

    nc.finalize()
    return nc


def _get_nc():
    if "nc" not in _CACHE:
        _CACHE["nc"] = _build_nc()
    return _CACHE["nc"]


def kernel(x, y, w_qkv, w_out, b_out):
    from concourse.bass_utils import run_bass_kernel_spmd

    nc = _get_nc()

    x = np.asarray(x, dtype=np.float32).reshape(16, C, N).astype(np.float16)
    y = np.asarray(y, dtype=np.float32).reshape(16, C, N).astype(np.float16)
    w_qkv = np.asarray(w_qkv, dtype=np.float32)
    wq_t = np.ascontiguousarray(w_qkv[0:HID].T).astype(np.float16)
    wk_t = np.ascontiguousarray(w_qkv[HID:2 * HID].T).astype(np.float16)
    wv_t = np.ascontiguousarray(w_qkv[2 * HID:3 * HID].T).astype(np.float16)
    wo_t = np.ascontiguousarray(np.asarray(w_out, dtype=np.float32).T).astype(np.float16)
    bo = np.ascontiguousarray(
        np.asarray(b_out, dtype=np.float32).reshape(2, 128, 1))

    in_maps = []
    for c in range(NCORES):
        in_maps.append({
            "x": np.ascontiguousarray(x[c * NB:(c + 1) * NB]),
            "y": np.ascontiguousarray(y[c * NB:(c + 1) * NB]),
            "wq_t": wq_t, "wk_t": wk_t, "wv_t": wv_t, "wo_t": wo_t,
            "b_out": bo,
        })

    res = run_bass_kernel_spmd(nc, in_maps, list(range(NCORES)))
    full = np.concatenate(
        [res.results[i]["out"].astype(np.float32) for i in range(NCORES)], axis=0)
    return full.reshape(16, C, 32, 32)
